# revision 39
# baseline (speedup 1.0000x reference)
"""Gated multi-head self-attention on 8 Trainium2 NeuronCores.

Sharding: batch (B=2) x head-groups (4 groups of 4 heads) -> 8 cores.
Each core computes, for its batch b and its 4 heads:
    partial_out[t, e] = sum_h gate[h] * (softmax(Q_h K_h^T / 8) (V_h + bv_h) Wo_h)
The host sums the 4 head-group partials per batch, adds sum_h gate_h*bo_h,
and stacks the two batches.

All matmuls bf16 (full-rate moving operands); PSUM accumulation fp32.
Schedule is ACT(exp)-bound: per (t-chunk, head-pair) block, 16 s-tiles of
scoresT = K^T Q (row-tiled matmul pairs) -> exp (ACT, bf16 out) -> rowsum
ping-pong adds (DVE) + PV col-tiled pairs (PE, PSUM accum).  The PV pair is
emitted one s-tile late so it never head-of-line-blocks the next scores pair
in the strict-FIFO PE queue.  Block tails (rowsum broadcast mask-matmul,
reciprocal_approx_fast, ctx normalize + bv), output-projection tiles and
next-chunk Q projections are split into small items and woven, evenly paced,
into the FOLLOWING block's loop so the scalar engine never starves between
blocks.  Only K(pr0,tch0)+Q(tch0,pr0) run as a serial prologue; the other
K/Q tiles are woven into the first block, V tiles hide behind the hT DMA
and the first block.  Weights DMA before hT; hT lands as 16 [P,1024]
chunks.  bq/bk fold into the ACT/DVE PSUM->SBUF copies, bv into the tail
normalize (per-partition tensor_scalar), bo is added on the host.
"""

import numpy as np
import ml_dtypes
from collections import deque
from contextlib import ExitStack

import concourse.bass as bass
import concourse.tile as tile
from concourse import bacc, mybir
from concourse import bass_utils

E, H, D = 1024, 16, 64
B, T = 2, 2048
NCORES = 8
P = 128
TC = 512          # t-chunk (PSUM bank = 512 fp32)
NTC = T // TC     # 4 t-chunks
NST = T // P      # 16 s-tiles
NEC = E // P      # 8 e-chunks

F32 = mybir.dt.float32
BF16 = mybir.dt.bfloat16
BF = ml_dtypes.bfloat16
AF = mybir.ActivationFunctionType


def build_kernel():
    nc = bacc.Bacc("TRN2", target_bir_lowering=False, debug=False,
                   num_devices=NCORES)
    hT = nc.dram_tensor("hT", [P, NEC, T], BF16, kind="ExternalInput").ap()
    wq = nc.dram_tensor("wq", [P, 2, NEC, P], BF16, kind="ExternalInput").ap()
    wk = nc.dram_tensor("wk", [P, 2, NEC, P], BF16, kind="ExternalInput").ap()
    wv = nc.dram_tensor("wv", [P, NEC, 256], BF16, kind="ExternalInput").ap()
    wo = nc.dram_tensor("wo", [P, 2, E], BF16, kind="ExternalInput").ap()
    bq = nc.dram_tensor("bq", [P, 2], F32, kind="ExternalInput").ap()
    bk = nc.dram_tensor("bk", [P, 2], F32, kind="ExternalInput").ap()
    bvp = nc.dram_tensor("bvp", [P, 2], F32, kind="ExternalInput").ap()
    mask = nc.dram_tensor("mask", [P, 2 * P], BF16, kind="ExternalInput").ap()
    out = nc.dram_tensor("out", [T, E], BF16, kind="ExternalOutput").ap()

    with tile.TileContext(nc) as tc:
        with ExitStack() as ctx:
            persist = ctx.enter_context(tc.tile_pool(name="persist", bufs=1))
            work = ctx.enter_context(tc.tile_pool(name="work", bufs=6))
            rspool = ctx.enter_context(tc.tile_pool(name="rspool", bufs=6))
            rpool = ctx.enter_context(tc.tile_pool(name="rpool", bufs=2))
            opool = ctx.enter_context(tc.tile_pool(name="opool", bufs=4))
            ps_s = ctx.enter_context(tc.tile_pool(name="ps_s", bufs=2, space="PSUM"))
            ps_ctx = ctx.enter_context(tc.tile_pool(name="ps_ctx", bufs=2, space="PSUM"))
            ps_misc = ctx.enter_context(tc.tile_pool(name="ps_misc", bufs=2, space="PSUM"))

            # ---- persistent SBUF tensors ----
            hT_sb = persist.tile([P, NEC, T], BF16, tag="hT")
            wq_sb = persist.tile([P, 2 * NEC * P], BF16, tag="wq")
            wk_sb = persist.tile([P, 2 * NEC * P], BF16, tag="wk")
            wv_sb = persist.tile([P, NEC, 256], BF16, tag="wv")
            wo_sb = persist.tile([P, 2, E], BF16, tag="wo")
            bq_sb = persist.tile([P, 2], F32, tag="bq")
            bk_sb = persist.tile([P, 2], F32, tag="bk")
            bvp_sb = persist.tile([P, 2], F32, tag="bvp")
            mask_sb = persist.tile([P, 2 * P], BF16, tag="mask")
            QT_sb = persist.tile([P, 2, T], BF16, tag="QT")
            KT_sb = persist.tile([P, 2, T], BF16, tag="KT")
            V_sb = persist.tile([P, NST, 256], BF16, tag="V")
            ctx_sb = persist.tile([P, 2, T], BF16, tag="ctx")

            with nc.named_scope("load"):
                # mask first: it is tiny and feeds the PE warm-up filler
                nc.sync.dma_start(mask_sb[:], mask)
                # weights next: every projection is gated on them, and they
                # are small next to hT
                nc.sync.dma_start(wk_sb[:], wk)
                nc.sync.dma_start(bk_sb[:], bk)
                nc.sync.dma_start(wq_sb[:], wq)
                nc.sync.dma_start(bq_sb[:], bq)
                nc.sync.dma_start(wv_sb[:], wv)
                nc.sync.dma_start(bvp_sb[:], bvp)
                # hT in [P,1024] chunks (2KB/partition rows), t-half 0 first
                for th in range(2):
                    for ec in range(NEC):
                        nc.sync.dma_start(
                            hT_sb[:, ec, th * 1024:(th + 1) * 1024],
                            hT[:, ec, th * 1024:(th + 1) * 1024])
                nc.sync.dma_start(wo_sb[:], wo)

            def qk_proj(w_sb, b_sb, dst, pr, tch, on_act):
                t0 = tch * TC
                ps = ps_misc.tile([P, TC], F32, tag="ps_misc")
                for ec in range(NEC):
                    w0 = (pr * NEC + ec) * P
                    nc.tensor.matmul(ps[:], w_sb[:, w0:w0 + P],
                                     hT_sb[:, ec, t0:t0 + TC],
                                     start=(ec == 0), stop=(ec == NEC - 1))
                if on_act:
                    nc.scalar.activation(dst[:, pr, t0:t0 + TC], ps[:],
                                         AF.Identity, bias=b_sb[:, pr:pr + 1],
                                         scale=1.0)
                else:
                    nc.vector.tensor_scalar(dst[:, pr, t0:t0 + TC], ps[:],
                                            b_sb[:, pr:pr + 1], None,
                                            mybir.AluOpType.add)

            def qk_proj_items(w_sb, b_sb, dst, pr, tch, chunk=2):
                """qk_proj split into small deferred items (`chunk` MMs each +
                a DVE bias-copy) so no single item delays the score matmuls
                of the block it's woven into.  Items of one tile must all be
                emitted within ~1 loop iteration of each other (chunk>=4 when
                the host block also allocates ps_misc tiles inline) so the
                tile's ps_misc slot frees before the pool wraps around."""
                t0 = tch * TC
                box = {}

                def mm_chunk(e0):
                    def emit():
                        if e0 == 0:
                            box["ps"] = ps_misc.tile([P, TC], F32,
                                                     name="qp_ps",
                                                     tag="ps_misc")
                        ps = box["ps"]
                        for ec in range(e0, e0 + chunk):
                            w0 = (pr * NEC + ec) * P
                            nc.tensor.matmul(ps[:], w_sb[:, w0:w0 + P],
                                             hT_sb[:, ec, t0:t0 + TC],
                                             start=(ec == 0),
                                             stop=(ec == NEC - 1),
                                             skip_group_check=True)
                    return emit

                def copy():
                    nc.vector.tensor_scalar(dst[:, pr, t0:t0 + TC],
                                            box["ps"][:],
                                            b_sb[:, pr:pr + 1], None,
                                            mybir.AluOpType.add)
                return [mm_chunk(e0) for e0 in range(0, NEC, chunk)] + [copy]

            def v_proj(st):
                ps = ps_misc.tile([P, TC], F32, tag="ps_misc")
                psv = ps[:, :256]
                for ec in range(NEC):
                    nc.tensor.matmul(psv, hT_sb[:, ec, st * P:(st + 1) * P],
                                     wv_sb[:, ec, :], start=(ec == 0),
                                     stop=(ec == NEC - 1))
                nc.vector.tensor_copy(V_sb[:, st, :], psv)

            def attn_block(tch, pr, merge_v, deferred):
                t0 = tch * TC
                c0 = pr * P
                pctx = ps_ctx.tile([P, TC], F32, tag="ps_ctx")
                rs0 = rspool.tile([P, 2 * TC], BF16, tag="rs")
                rs1 = rspool.tile([P, 2 * TC], BF16, tag="rs")
                rstiles = (rs0, rs1)

                def pv(st, ex):
                    nc.tensor.matmul(
                        pctx[0:64, :], V_sb[:, st, c0:c0 + 64],
                        ex[:, :TC],
                        start=(st == 0), stop=(st == NST - 1),
                        tile_position=(0, 0), skip_group_check=True)
                    nc.tensor.matmul(
                        pctx[64:P, :], V_sb[:, st, c0 + 64:c0 + P],
                        ex[:, TC:],
                        start=(st == 0), stop=(st == NST - 1),
                        tile_position=(0, 64), skip_group_check=True)

                prev = None
                for st in range(NST):
                    if merge_v and st >= 8:
                        v_proj(st)
                    s0 = st * P
                    pss = ps_s.tile([P, 2 * TC], F32, tag="ps_s")
                    nc.tensor.matmul(
                        pss[:, :TC], KT_sb[0:64, pr, s0:s0 + P],
                        QT_sb[0:64, pr, t0:t0 + TC],
                        start=True, stop=True, tile_position=(0, 0))
                    nc.tensor.matmul(
                        pss[:, TC:], KT_sb[64:P, pr, s0:s0 + P],
                        QT_sb[64:P, pr, t0:t0 + TC],
                        start=True, stop=True, tile_position=(64, 0))
                    ex = work.tile([P, 2 * TC], BF16, tag="expT")
                    nc.scalar.activation(ex[:], pss[:], AF.Exp, scale=0.125)
                    if st == 1:
                        # first two exp tiles summed directly: no init copy
                        nc.vector.tensor_add(rs1[:], prev[1][:], ex[:])
                    elif st > 1:
                        nc.vector.tensor_add(rstiles[st % 2][:],
                                             rstiles[(st + 1) % 2][:], ex[:])
                    # PV for the PREVIOUS s-tile: keeps the next scores pair
                    # ahead of the PV matmul that stalls on exp(st) in the
                    # strict-FIFO PE queue.
                    if prev is not None:
                        pv(*prev)
                    # drain deferred work at an even pace across the block
                    n = -(-len(deferred) // (NST - st))  # ceil
                    for _ in range(min(n, len(deferred))):
                        deferred.popleft()()
                    prev = (st, ex)
                pv(*prev)
                while deferred:
                    deferred.popleft()()
                rs_fin = rstiles[(NST - 1) % 2]

                box = {}

                def t_mask():
                    # col-tiled pair: head-A rowsum broadcast to rows 0-63,
                    # head-B to rows 64-127, concurrently
                    pR = ps_misc.tile([P, TC], F32, tag="ps_misc")
                    nc.tensor.matmul(pR[0:64, :], mask_sb[:, 0:64],
                                     rs_fin[:, :TC],
                                     start=True, stop=True,
                                     tile_position=(0, 0),
                                     skip_group_check=True)
                    nc.tensor.matmul(pR[64:P, :], mask_sb[:, P + 64:2 * P],
                                     rs_fin[:, TC:],
                                     start=True, stop=True,
                                     tile_position=(0, 64),
                                     skip_group_check=True)
                    box["pR"] = pR

                def t_recip():
                    R_sb = rpool.tile([P, TC], F32, tag="R")
                    with nc.allow_low_precision(reason="~51-ULP recip is plenty for softmax denom"):
                        nc.vector.reciprocal_approx_fast(R_sb[:], box["pR"][:])
                    box["R"] = R_sb

                def t_mult():
                    cslice = ctx_sb[:, pr, t0:t0 + TC]
                    nc.vector.tensor_tensor(
                        cslice, pctx[:], box["R"][:], mybir.AluOpType.mult)
                    nc.vector.tensor_scalar(cslice, cslice,
                                            bvp_sb[:, pr:pr + 1], None,
                                            mybir.AluOpType.add)
                return [t_mask, t_recip, t_mult]

            def outproj_unit(tt, ec2, on_act=False, pso_ap=None):
                box = {}

                def mms():
                    pso = (pso_ap if pso_ap is not None
                           else ps_misc.tile([P, TC], F32, name="op_ps",
                                             tag="ps_misc"))
                    for pr in range(2):
                        nc.tensor.matmul(
                            pso[:], ctx_sb[:, pr, tt * P:(tt + 1) * P],
                            wo_sb[:, pr, ec2 * TC:(ec2 + 1) * TC],
                            start=(pr == 0), stop=(pr == 1),
                            skip_group_check=True)
                    box["pso"] = pso

                def store():
                    o_sb = opool.tile([P, TC], BF16, tag="o")
                    if on_act:
                        nc.scalar.copy(o_sb[:], box["pso"][:])
                    else:
                        nc.vector.tensor_copy(o_sb[:], box["pso"][:])
                    nc.sync.dma_start(
                        out[tt * P:(tt + 1) * P, ec2 * TC:(ec2 + 1) * TC],
                        o_sb[:])
                return [mms, store]

            with nc.named_scope("qkv"):
                # HAM warm-up: ~120 no-output matmuls on the early-arriving
                # mask tile keep the PE at full clock through the hT DMA
                # window, so the projection chain runs warm from its first MM
                warm = ps_misc.tile([P, TC], F32, name="warm", tag="ps_misc")
                for _ in range(120):
                    nc.tensor.matmul(warm[:, :256], mask_sb[:, 0:P],
                                     mask_sb[:, 0:256], start=True, stop=True,
                                     skip_group_check=True)
                qk_proj(wk_sb, bk_sb, KT_sb, 0, 0, on_act=True)
                qk_proj(wq_sb, bq_sb, QT_sb, 0, 0, on_act=True)
                # V tiles of the first t-half: their inputs arrive with the
                # early hT chunks, so they hide under the tail of the DMA
                for st in range(8):
                    v_proj(st)

            with nc.named_scope("attn"):
                tails = {}
                for tch in range(NTC):
                    for pr in range(2):
                        d = deque()
                        if (tch, pr) == (0, 0):
                            # rest of K plus Q(0, pr1), woven into the loop;
                            # K(pr0, sc) items land well before scores need
                            # them at st = 4*sc (even-paced draining).
                            for sc in range(1, NTC):
                                d.extend(qk_proj_items(
                                    wk_sb, bk_sb, KT_sb, 0, sc, chunk=4))
                            for sc in range(NTC):
                                d.extend(qk_proj_items(
                                    wk_sb, bk_sb, KT_sb, 1, sc, chunk=4))
                            d.extend(qk_proj_items(
                                wq_sb, bq_sb, QT_sb, 1, 0, chunk=4))
                        elif pr == 0 and tch > 0:
                            # balanced PE load: half of outproj(tch-1) +
                            # Q(tch, pr1) (Q(tch, pr0) was emitted a block
                            # earlier)
                            d.extend(tails[(tch - 1, 1)])
                            for tt in range((tch - 1) * 4, (tch - 1) * 4 + 2):
                                for ec2 in range(2):
                                    d.extend(outproj_unit(tt, ec2))
                            d.extend(qk_proj_items(
                                wq_sb, bq_sb, QT_sb, 1, tch))
                        elif pr == 1:
                            d.extend(tails[(tch, 0)])
                            if tch > 0:
                                for tt in range((tch - 1) * 4 + 2,
                                                (tch - 1) * 4 + 4):
                                    for ec2 in range(2):
                                        d.extend(outproj_unit(tt, ec2))
                            if tch < NTC - 1:
                                d.extend(qk_proj_items(
                                    wq_sb, bq_sb, QT_sb, 0, tch + 1))
                        tails[(tch, pr)] = attn_block(
                            tch, pr, merge_v=(tch == 0 and pr == 0),
                            deferred=d)
                with nc.named_scope("outproj"):
                    for item in tails[(NTC - 1, 1)]:
                        item()
                    # the score banks are free after the last exp: borrow
                    # them so 6 projection tiles are in flight at once, and
                    # route half the final PSUM->SBUF copies to the idle ACT
                    big0 = ps_s.tile([P, 2 * TC], F32, name="ob0", tag="ps_s")
                    big1 = ps_s.tile([P, 2 * TC], F32, name="ob1", tag="ps_s")
                    psos = [big0[:, :TC], big0[:, TC:],
                            big1[:, :TC], big1[:, TC:],
                            None, None, None, None]
                    units = []
                    k = 0
                    for tt in range((NTC - 1) * 4, NTC * 4):
                        for ec2 in range(2):
                            units.append(outproj_unit(
                                tt, ec2, on_act=(k % 2 == 1),
                                pso_ap=psos[k]))
                            k += 1
                    for mms, _ in units:
                        mms()
                    for _, store in units:
                        store()
    nc.compile()
    return nc


_NC = None


def _get_nc():
    global _NC
    if _NC is None:
        _NC = build_kernel()
    return _NC


def make_in_maps(hidden_states, Wq, bq, Wk, bk, Wv, bv, Wo, bo, gate):
    f = np.float32
    hidden_states = np.asarray(hidden_states, f)
    Wq, bq = np.asarray(Wq, f), np.asarray(bq, f)
    Wk, bk = np.asarray(Wk, f), np.asarray(bk, f)
    Wv, bv = np.asarray(Wv, f), np.asarray(bv, f)
    Wo, bo = np.asarray(Wo, f), np.asarray(bo, f)
    gate = np.asarray(gate, f)

    # [P, NEC, T] bf16 per batch
    hT_b = [np.ascontiguousarray(
                hidden_states[b].T.reshape(NEC, P, T).transpose(1, 0, 2)
            ).astype(BF) for b in range(B)]
    mask_np = np.zeros((P, 2 * P), f)
    mask_np[:, 0:64] = 1.0        # maskA: broadcast head-A rowsum to rows 0-63
    mask_np[:, P + 64:2 * P] = 1.0  # maskB: head-B rowsum to rows 64-127
    mask_np = mask_np.astype(BF)

    in_maps = []
    for core in range(NCORES):
        b, hg = divmod(core, 4)
        hs = [4 * hg + i for i in range(4)]

        def pack_qk(W):
            outw = np.empty((2, NEC, P, P), f)
            for pr in range(2):
                pair = np.concatenate(
                    [W[hs[2 * pr]], W[hs[2 * pr + 1]]], axis=1)  # [E, 128]
                outw[pr] = pair.reshape(NEC, P, P)
            # -> [P(e-part), 2, NEC, P(d-pair)]
            return np.ascontiguousarray(outw.transpose(2, 0, 1, 3)).astype(BF)

        def pack_b(bx):
            o = np.empty((P, 2), f)
            for pr in range(2):
                o[:, pr] = np.concatenate([bx[hs[2 * pr]], bx[hs[2 * pr + 1]]])
            return np.ascontiguousarray(o)

        wv_np = np.concatenate([Wv[h] for h in hs], axis=1)  # [E, 256]
        wv_np = np.ascontiguousarray(
            wv_np.reshape(NEC, P, 256).transpose(1, 0, 2)).astype(BF)
        wo_np = np.empty((2, P, E), f)
        for pr in range(2):
            h0, h1 = hs[2 * pr], hs[2 * pr + 1]
            wo_np[pr] = np.concatenate(
                [gate[h0] * Wo[h0], gate[h1] * Wo[h1]], axis=0)  # [128, E]
        wo_np = np.ascontiguousarray(wo_np.transpose(1, 0, 2)).astype(BF)
        in_maps.append(dict(
            hT=hT_b[b],
            wq=pack_qk(Wq), wk=pack_qk(Wk),
            wv=wv_np, wo=wo_np,
            bq=pack_b(bq), bk=pack_b(bk), bvp=pack_b(bv),
            mask=mask_np,
        ))
    bo_sum = (gate[:, None] * bo).sum(axis=0).astype(f)  # [E]
    return in_maps, bo_sum


def kernel(hidden_states, Wq, bq, Wk, bk, Wv, bv, Wo, bo, gate, _trace=False,
           **run_kwargs):
    nc = _get_nc()
    in_maps, bo_sum = make_in_maps(
        hidden_states, Wq, bq, Wk, bk, Wv, bv, Wo, bo, gate)
    res = bass_utils.run_bass_kernel_spmd(
        nc, in_maps, core_ids=list(range(NCORES)), trace=_trace, **run_kwargs)
    outs = [np.asarray(r["out"], np.float32) for r in res.results]
    full = np.stack([
        outs[0] + outs[1] + outs[2] + outs[3] + bo_sum,
        outs[4] + outs[5] + outs[6] + outs[7] + bo_sum,
    ]).astype(np.float32)
    kernel.last_result = res
    return full


# revision 40
# speedup vs baseline: 1.0199x; 1.0199x over previous
"""Gated multi-head self-attention on 8 Trainium2 NeuronCores.

Sharding: batch (B=2) x head-groups (4 groups of 4 heads) -> 8 cores.
Each core computes, for its batch b and its 4 heads:
    partial_out[t, e] = sum_h gate[h] * (softmax(Q_h K_h^T / 8) (V_h + bv_h) Wo_h)
The host sums the 4 head-group partials per batch, adds sum_h gate_h*bo_h,
and stacks the two batches.

All matmuls bf16 (full-rate moving operands); PSUM accumulation fp32.
Schedule is ACT(exp)-bound: per (t-chunk, head-pair) block, 16 s-tiles of
scoresT = K^T Q (row-tiled matmul pairs) -> exp (ACT, bf16 out) -> rowsum
ping-pong adds (DVE) + PV col-tiled pairs (PE, PSUM accum).  The PV pair is
emitted one s-tile late so it never head-of-line-blocks the next scores pair
in the strict-FIFO PE queue.  Block tails (rowsum broadcast mask-matmul,
reciprocal_approx_fast, ctx normalize + bv), output-projection tiles and
next-chunk Q projections are split into small items and woven, evenly paced,
into the FOLLOWING block's loop so the scalar engine never starves between
blocks.  Only K(pr0,tch0)+Q(tch0,pr0) run as a serial prologue; the other
K/Q tiles are woven into the first block, V tiles hide behind the hT DMA
and the first block.  Weights DMA before hT; hT lands as 16 [P,1024]
chunks.  bq/bk fold into the ACT/DVE PSUM->SBUF copies, bv into the tail
normalize (per-partition tensor_scalar), bo is added on the host.
"""

import numpy as np
import ml_dtypes
from collections import deque
from contextlib import ExitStack

import concourse.bass as bass
import concourse.tile as tile
from concourse import bacc, mybir
from concourse import bass_utils

E, H, D = 1024, 16, 64
B, T = 2, 2048
NCORES = 8
P = 128
TC = 512          # t-chunk (PSUM bank = 512 fp32)
NTC = T // TC     # 4 t-chunks
NST = T // P      # 16 s-tiles
NEC = E // P      # 8 e-chunks

F32 = mybir.dt.float32
BF16 = mybir.dt.bfloat16
BF = ml_dtypes.bfloat16
AF = mybir.ActivationFunctionType


def build_kernel():
    nc = bacc.Bacc("TRN2", target_bir_lowering=False, debug=False,
                   num_devices=NCORES)
    hT = nc.dram_tensor("hT", [P, NEC, T], BF16, kind="ExternalInput").ap()
    wq = nc.dram_tensor("wq", [P, 2, NEC, P], BF16, kind="ExternalInput").ap()
    wk = nc.dram_tensor("wk", [P, 2, NEC, P], BF16, kind="ExternalInput").ap()
    wv = nc.dram_tensor("wv", [P, NEC, 256], BF16, kind="ExternalInput").ap()
    wo = nc.dram_tensor("wo", [P, 2, E], BF16, kind="ExternalInput").ap()
    bq = nc.dram_tensor("bq", [P, 2], F32, kind="ExternalInput").ap()
    bk = nc.dram_tensor("bk", [P, 2], F32, kind="ExternalInput").ap()
    bvp = nc.dram_tensor("bvp", [P, 2], F32, kind="ExternalInput").ap()
    mask = nc.dram_tensor("mask", [P, 2 * P], BF16, kind="ExternalInput").ap()
    out = nc.dram_tensor("out", [T, E], BF16, kind="ExternalOutput").ap()

    with tile.TileContext(nc) as tc:
        with ExitStack() as ctx:
            persist = ctx.enter_context(tc.tile_pool(name="persist", bufs=1))
            work = ctx.enter_context(tc.tile_pool(name="work", bufs=6))
            rspool = ctx.enter_context(tc.tile_pool(name="rspool", bufs=6))
            rpool = ctx.enter_context(tc.tile_pool(name="rpool", bufs=2))
            opool = ctx.enter_context(tc.tile_pool(name="opool", bufs=4))
            ps_s = ctx.enter_context(tc.tile_pool(name="ps_s", bufs=2, space="PSUM"))
            ps_ctx = ctx.enter_context(tc.tile_pool(name="ps_ctx", bufs=2, space="PSUM"))
            ps_misc = ctx.enter_context(tc.tile_pool(name="ps_misc", bufs=2, space="PSUM"))

            # ---- persistent SBUF tensors ----
            hT_sb = persist.tile([P, NEC, T], BF16, tag="hT")
            wq_sb = persist.tile([P, 2 * NEC * P], BF16, tag="wq")
            wk_sb = persist.tile([P, 2 * NEC * P], BF16, tag="wk")
            wv_sb = persist.tile([P, NEC, 256], BF16, tag="wv")
            wo_sb = persist.tile([P, 2, E], BF16, tag="wo")
            bq_sb = persist.tile([P, 2], F32, tag="bq")
            bk_sb = persist.tile([P, 2], F32, tag="bk")
            bvp_sb = persist.tile([P, 2], F32, tag="bvp")
            mask_sb = persist.tile([P, 2 * P], BF16, tag="mask")
            QT_sb = persist.tile([P, 2, T], BF16, tag="QT")
            KT_sb = persist.tile([P, 2, T], BF16, tag="KT")
            V_sb = persist.tile([P, NST, 256], BF16, tag="V")
            ctx_sb = persist.tile([P, 2, T], BF16, tag="ctx")

            with nc.named_scope("load"):
                # weights first: every projection is gated on them, and they
                # are small next to hT
                nc.sync.dma_start(wk_sb[:], wk)
                nc.sync.dma_start(bk_sb[:], bk)
                nc.sync.dma_start(wq_sb[:], wq)
                nc.sync.dma_start(bq_sb[:], bq)
                nc.sync.dma_start(wv_sb[:], wv)
                nc.sync.dma_start(bvp_sb[:], bvp)
                nc.sync.dma_start(mask_sb[:], mask)
                # hT in [P,1024] chunks (2KB/partition rows), t-half 0 first
                for th in range(2):
                    for ec in range(NEC):
                        nc.sync.dma_start(
                            hT_sb[:, ec, th * 1024:(th + 1) * 1024],
                            hT[:, ec, th * 1024:(th + 1) * 1024])
                nc.sync.dma_start(wo_sb[:], wo)

            def qk_proj(w_sb, b_sb, dst, pr, tch, on_act):
                t0 = tch * TC
                ps = ps_misc.tile([P, TC], F32, tag="ps_misc")
                for ec in range(NEC):
                    w0 = (pr * NEC + ec) * P
                    nc.tensor.matmul(ps[:], w_sb[:, w0:w0 + P],
                                     hT_sb[:, ec, t0:t0 + TC],
                                     start=(ec == 0), stop=(ec == NEC - 1))
                if on_act:
                    nc.scalar.activation(dst[:, pr, t0:t0 + TC], ps[:],
                                         AF.Identity, bias=b_sb[:, pr:pr + 1],
                                         scale=1.0)
                else:
                    nc.vector.tensor_scalar(dst[:, pr, t0:t0 + TC], ps[:],
                                            b_sb[:, pr:pr + 1], None,
                                            mybir.AluOpType.add)

            def qk_proj_items(w_sb, b_sb, dst, pr, tch, chunk=2):
                """qk_proj split into small deferred items (`chunk` MMs each +
                a DVE bias-copy) so no single item delays the score matmuls
                of the block it's woven into.  Items of one tile must all be
                emitted within ~1 loop iteration of each other (chunk>=4 when
                the host block also allocates ps_misc tiles inline) so the
                tile's ps_misc slot frees before the pool wraps around."""
                t0 = tch * TC
                box = {}

                def mm_chunk(e0):
                    def emit():
                        if e0 == 0:
                            box["ps"] = ps_misc.tile([P, TC], F32,
                                                     name="qp_ps",
                                                     tag="ps_misc")
                        ps = box["ps"]
                        for ec in range(e0, e0 + chunk):
                            w0 = (pr * NEC + ec) * P
                            nc.tensor.matmul(ps[:], w_sb[:, w0:w0 + P],
                                             hT_sb[:, ec, t0:t0 + TC],
                                             start=(ec == 0),
                                             stop=(ec == NEC - 1),
                                             skip_group_check=True)
                    return emit

                def copy():
                    nc.vector.tensor_scalar(dst[:, pr, t0:t0 + TC],
                                            box["ps"][:],
                                            b_sb[:, pr:pr + 1], None,
                                            mybir.AluOpType.add)
                return [mm_chunk(e0) for e0 in range(0, NEC, chunk)] + [copy]

            def v_proj(st):
                ps = ps_misc.tile([P, TC], F32, tag="ps_misc")
                psv = ps[:, :256]
                for ec in range(NEC):
                    nc.tensor.matmul(psv, hT_sb[:, ec, st * P:(st + 1) * P],
                                     wv_sb[:, ec, :], start=(ec == 0),
                                     stop=(ec == NEC - 1))
                nc.vector.tensor_copy(V_sb[:, st, :], psv)

            def attn_block(tch, pr, merge_v, deferred):
                t0 = tch * TC
                c0 = pr * P
                pctx = ps_ctx.tile([P, TC], F32, tag="ps_ctx")
                rs0 = rspool.tile([P, 2 * TC], BF16, tag="rs")
                rs1 = rspool.tile([P, 2 * TC], BF16, tag="rs")
                rstiles = (rs0, rs1)

                def pv(st, ex):
                    nc.tensor.matmul(
                        pctx[0:64, :], V_sb[:, st, c0:c0 + 64],
                        ex[:, :TC],
                        start=(st == 0), stop=(st == NST - 1),
                        tile_position=(0, 0), skip_group_check=True)
                    nc.tensor.matmul(
                        pctx[64:P, :], V_sb[:, st, c0 + 64:c0 + P],
                        ex[:, TC:],
                        start=(st == 0), stop=(st == NST - 1),
                        tile_position=(0, 64), skip_group_check=True)

                prev = None
                for st in range(NST):
                    if merge_v and st >= 8:
                        v_proj(st)
                    s0 = st * P
                    pss = ps_s.tile([P, 2 * TC], F32, tag="ps_s")
                    nc.tensor.matmul(
                        pss[:, :TC], KT_sb[0:64, pr, s0:s0 + P],
                        QT_sb[0:64, pr, t0:t0 + TC],
                        start=True, stop=True, tile_position=(0, 0))
                    nc.tensor.matmul(
                        pss[:, TC:], KT_sb[64:P, pr, s0:s0 + P],
                        QT_sb[64:P, pr, t0:t0 + TC],
                        start=True, stop=True, tile_position=(64, 0))
                    ex = work.tile([P, 2 * TC], BF16, tag="expT")
                    nc.scalar.activation(ex[:], pss[:], AF.Exp, scale=0.125)
                    if st == 1:
                        # first two exp tiles summed directly: no init copy
                        nc.vector.tensor_add(rs1[:], prev[1][:], ex[:])
                    elif st > 1:
                        nc.vector.tensor_add(rstiles[st % 2][:],
                                             rstiles[(st + 1) % 2][:], ex[:])
                    # PV for the PREVIOUS s-tile: keeps the next scores pair
                    # ahead of the PV matmul that stalls on exp(st) in the
                    # strict-FIFO PE queue.
                    if prev is not None:
                        pv(*prev)
                    # drain deferred work at an even pace across the block
                    n = -(-len(deferred) // (NST - st))  # ceil
                    for _ in range(min(n, len(deferred))):
                        deferred.popleft()()
                    prev = (st, ex)
                pv(*prev)
                while deferred:
                    deferred.popleft()()
                rs_fin = rstiles[(NST - 1) % 2]

                box = {}

                def t_mask():
                    # col-tiled pair: head-A rowsum broadcast to rows 0-63,
                    # head-B to rows 64-127, concurrently
                    pR = ps_misc.tile([P, TC], F32, tag="ps_misc")
                    nc.tensor.matmul(pR[0:64, :], mask_sb[:, 0:64],
                                     rs_fin[:, :TC],
                                     start=True, stop=True,
                                     tile_position=(0, 0),
                                     skip_group_check=True)
                    nc.tensor.matmul(pR[64:P, :], mask_sb[:, P + 64:2 * P],
                                     rs_fin[:, TC:],
                                     start=True, stop=True,
                                     tile_position=(0, 64),
                                     skip_group_check=True)
                    box["pR"] = pR

                def t_recip():
                    R_sb = rpool.tile([P, TC], F32, tag="R")
                    with nc.allow_low_precision(reason="~51-ULP recip is plenty for softmax denom"):
                        nc.vector.reciprocal_approx_fast(R_sb[:], box["pR"][:])
                    box["R"] = R_sb

                def t_mult():
                    cslice = ctx_sb[:, pr, t0:t0 + TC]
                    nc.vector.tensor_tensor(
                        cslice, pctx[:], box["R"][:], mybir.AluOpType.mult)
                    nc.vector.tensor_scalar(cslice, cslice,
                                            bvp_sb[:, pr:pr + 1], None,
                                            mybir.AluOpType.add)
                return [t_mask, t_recip, t_mult]

            def outproj_unit(tt, ec2, on_act=False, pso_ap=None):
                box = {}

                def mms():
                    pso = (pso_ap if pso_ap is not None
                           else ps_misc.tile([P, TC], F32, name="op_ps",
                                             tag="ps_misc"))
                    for pr in range(2):
                        nc.tensor.matmul(
                            pso[:], ctx_sb[:, pr, tt * P:(tt + 1) * P],
                            wo_sb[:, pr, ec2 * TC:(ec2 + 1) * TC],
                            start=(pr == 0), stop=(pr == 1),
                            skip_group_check=True)
                    box["pso"] = pso

                def store():
                    o_sb = opool.tile([P, TC], BF16, tag="o")
                    if on_act:
                        nc.scalar.copy(o_sb[:], box["pso"][:])
                    else:
                        nc.vector.tensor_copy(o_sb[:], box["pso"][:])
                    nc.sync.dma_start(
                        out[tt * P:(tt + 1) * P, ec2 * TC:(ec2 + 1) * TC],
                        o_sb[:])
                return [mms, store]

            with nc.named_scope("qkv"):
                qk_proj(wk_sb, bk_sb, KT_sb, 0, 0, on_act=True)
                qk_proj(wq_sb, bq_sb, QT_sb, 0, 0, on_act=True)
                # V tiles of the first t-half: their inputs arrive with the
                # early hT chunks, so they hide under the tail of the DMA
                for st in range(8):
                    v_proj(st)

            with nc.named_scope("attn"):
                tails = {}
                for tch in range(NTC):
                    for pr in range(2):
                        d = deque()
                        if (tch, pr) == (0, 0):
                            # rest of K plus Q(0, pr1), woven into the loop;
                            # K(pr0, sc) items land well before scores need
                            # them at st = 4*sc (even-paced draining).
                            for sc in range(1, NTC):
                                d.extend(qk_proj_items(
                                    wk_sb, bk_sb, KT_sb, 0, sc, chunk=4))
                            for sc in range(NTC):
                                d.extend(qk_proj_items(
                                    wk_sb, bk_sb, KT_sb, 1, sc, chunk=4))
                            d.extend(qk_proj_items(
                                wq_sb, bq_sb, QT_sb, 1, 0, chunk=4))
                        elif pr == 0 and tch > 0:
                            # balanced PE load: half of outproj(tch-1) +
                            # Q(tch, pr1) (Q(tch, pr0) was emitted a block
                            # earlier)
                            d.extend(tails[(tch - 1, 1)])
                            for tt in range((tch - 1) * 4, (tch - 1) * 4 + 2):
                                for ec2 in range(2):
                                    d.extend(outproj_unit(tt, ec2))
                            d.extend(qk_proj_items(
                                wq_sb, bq_sb, QT_sb, 1, tch))
                        elif pr == 1:
                            d.extend(tails[(tch, 0)])
                            if tch > 0:
                                for tt in range((tch - 1) * 4 + 2,
                                                (tch - 1) * 4 + 4):
                                    for ec2 in range(2):
                                        d.extend(outproj_unit(tt, ec2))
                            if tch < NTC - 1:
                                d.extend(qk_proj_items(
                                    wq_sb, bq_sb, QT_sb, 0, tch + 1))
                        tails[(tch, pr)] = attn_block(
                            tch, pr, merge_v=(tch == 0 and pr == 0),
                            deferred=d)
                with nc.named_scope("outproj"):
                    for item in tails[(NTC - 1, 1)]:
                        item()
                    # the score banks are free after the last exp: borrow
                    # them so 6 projection tiles are in flight at once, and
                    # route half the final PSUM->SBUF copies to the idle ACT
                    big0 = ps_s.tile([P, 2 * TC], F32, name="ob0", tag="ps_s")
                    big1 = ps_s.tile([P, 2 * TC], F32, name="ob1", tag="ps_s")
                    psos = [big0[:, :TC], big0[:, TC:],
                            big1[:, :TC], big1[:, TC:],
                            None, None, None, None]
                    units = []
                    k = 0
                    for tt in range((NTC - 1) * 4, NTC * 4):
                        for ec2 in range(2):
                            units.append(outproj_unit(
                                tt, ec2, on_act=(k % 2 == 1),
                                pso_ap=psos[k]))
                            k += 1
                    for mms, _ in units:
                        mms()
                    for _, store in units:
                        store()
    nc.compile()
    return nc


_NC = None


def _get_nc():
    global _NC
    if _NC is None:
        _NC = build_kernel()
    return _NC


def make_in_maps(hidden_states, Wq, bq, Wk, bk, Wv, bv, Wo, bo, gate):
    f = np.float32
    hidden_states = np.asarray(hidden_states, f)
    Wq, bq = np.asarray(Wq, f), np.asarray(bq, f)
    Wk, bk = np.asarray(Wk, f), np.asarray(bk, f)
    Wv, bv = np.asarray(Wv, f), np.asarray(bv, f)
    Wo, bo = np.asarray(Wo, f), np.asarray(bo, f)
    gate = np.asarray(gate, f)

    # [P, NEC, T] bf16 per batch
    hT_b = [np.ascontiguousarray(
                hidden_states[b].T.reshape(NEC, P, T).transpose(1, 0, 2)
            ).astype(BF) for b in range(B)]
    mask_np = np.zeros((P, 2 * P), f)
    mask_np[:, 0:64] = 1.0        # maskA: broadcast head-A rowsum to rows 0-63
    mask_np[:, P + 64:2 * P] = 1.0  # maskB: head-B rowsum to rows 64-127
    mask_np = mask_np.astype(BF)

    in_maps = []
    for core in range(NCORES):
        b, hg = divmod(core, 4)
        hs = [4 * hg + i for i in range(4)]

        def pack_qk(W):
            outw = np.empty((2, NEC, P, P), f)
            for pr in range(2):
                pair = np.concatenate(
                    [W[hs[2 * pr]], W[hs[2 * pr + 1]]], axis=1)  # [E, 128]
                outw[pr] = pair.reshape(NEC, P, P)
            # -> [P(e-part), 2, NEC, P(d-pair)]
            return np.ascontiguousarray(outw.transpose(2, 0, 1, 3)).astype(BF)

        def pack_b(bx):
            o = np.empty((P, 2), f)
            for pr in range(2):
                o[:, pr] = np.concatenate([bx[hs[2 * pr]], bx[hs[2 * pr + 1]]])
            return np.ascontiguousarray(o)

        wv_np = np.concatenate([Wv[h] for h in hs], axis=1)  # [E, 256]
        wv_np = np.ascontiguousarray(
            wv_np.reshape(NEC, P, 256).transpose(1, 0, 2)).astype(BF)
        wo_np = np.empty((2, P, E), f)
        for pr in range(2):
            h0, h1 = hs[2 * pr], hs[2 * pr + 1]
            wo_np[pr] = np.concatenate(
                [gate[h0] * Wo[h0], gate[h1] * Wo[h1]], axis=0)  # [128, E]
        wo_np = np.ascontiguousarray(wo_np.transpose(1, 0, 2)).astype(BF)
        in_maps.append(dict(
            hT=hT_b[b],
            wq=pack_qk(Wq), wk=pack_qk(Wk),
            wv=wv_np, wo=wo_np,
            bq=pack_b(bq), bk=pack_b(bk), bvp=pack_b(bv),
            mask=mask_np,
        ))
    bo_sum = (gate[:, None] * bo).sum(axis=0).astype(f)  # [E]
    return in_maps, bo_sum


def kernel(hidden_states, Wq, bq, Wk, bk, Wv, bv, Wo, bo, gate, _trace=False,
           **run_kwargs):
    nc = _get_nc()
    in_maps, bo_sum = make_in_maps(
        hidden_states, Wq, bq, Wk, bk, Wv, bv, Wo, bo, gate)
    res = bass_utils.run_bass_kernel_spmd(
        nc, in_maps, core_ids=list(range(NCORES)), trace=_trace, **run_kwargs)
    outs = [np.asarray(r["out"], np.float32) for r in res.results]
    full = np.stack([
        outs[0] + outs[1] + outs[2] + outs[3] + bo_sum,
        outs[4] + outs[5] + outs[6] + outs[7] + bo_sum,
    ]).astype(np.float32)
    kernel.last_result = res
    return full
